# revision 3
# baseline (speedup 1.0000x reference)
"""Trainium2 Bass kernel for nn_CrossEntropyLoss_2585570312585.

Reference computation (jax):
    cw = where(cw == 0, cw[0], cw)                      # [5]
    gold2dim   = argmax(gold, axis=class)               # [256,384]
    prediction = argmax(pred, axis=class)
    pred_fp    = where(gold2dim > 0, 0,
                       where(prediction == gold2dim, 0, prediction))
    weight_fp  = cw[pred_fp]
    loss = -(weight + weight_fp) * sum_c(gold * log(pred + 1e-8))
    out  = mean(loss)                                   # scalar

Restructuring (identical up to fp rounding / measure-zero ties):
    u      = sum_c gold_c * ln(pred_c + eps)
    gmask  = max(g1..g4) > g0            (== gold2dim > 0, first-max ties ok)
    cwsel  = sum_c cw_c * (p_c == max_c p_c)   (== cw[argmax p] up to ties)
    loss_pix = u * (w + cw0*gmask + (1-gmask)*cwsel)
    out = -mean(loss_pix)

Perf design (v3), from HW microbenchmarks of this toolchain:
  * ~16.7 us of the 19.3 us v1 baseline is fixed NEFF overhead. The
    controllable part: input-DMA latency, DVE compute, output-DMA tail.
  * All inputs bf16 (host converts; free for HW time). Measured end-to-end
    rel err of the full bf16 scheme: 2.0e-3 (gate 2e-2).
  * Inputs CLASS-BLOCKED per partition: [128, 5 blocks x 96]. Measured DVE
    costs: tensor_reduce 480->96 = 561 ns regardless of dtype (input-bound
    ~110 elem/ns), but plain bf16 tensor_tensor [128,96] = 116 ns. So every
    class reduction becomes a tree of contiguous TT ops (464 ns vs 561,
    and 348 vs 458+ for the 4-class max), and is_equal runs without the
    broadcast penalty path where possible.
  * tensor_tensor_reduce is broken on this toolchain (compiles, wedges the
    device: NRT_EXEC_UNIT_UNRECOVERABLE). Do not use.
  * Two parallel HWDGE queues: pred on qSP (Sync), gold+weight on
    qActivation (Scalar). No SWDGE (gpsimd) DMA.
  * cw baked as immediates into a class-blocked [128,480] bf16 pattern
    (5 contiguous memsets, issued pre-data -> free).
  * Output: plain [128,1] f32 DMA (PSEUDO_DMA_DIRECT2D desc-gen is ~650 ns
    fixed regardless of descriptor count; a TensorE partition-reduce to
    [1,1] just adds matmul+PSUM-copy latency). Host sums 128x8 partials.

Sharding: the 256x384 = 98304-pixel plane splits into 8 contiguous
chunks of 12288 pixels (one per NeuronCore), laid out [128 x 96] per
class block.
"""

import os
import sys

import numpy as np


def _ensure_concourse():
    try:
        import concourse  # noqa: F401
        return
    except ImportError:
        pass
    for p in ("/opt/trn_rl_repo", "/root/.axon_site/_ro/trn_rl_repo"):
        if os.path.isdir(p) and p not in sys.path:
            sys.path.insert(0, p)
    import concourse  # noqa: F401


_ensure_concourse()

import ml_dtypes  # noqa: E402

import concourse.bass as bass  # noqa: E402
import concourse.tile as tile  # noqa: E402
from concourse import bacc, mybir  # noqa: E402
from concourse.bass_utils import run_bass_kernel_spmd  # noqa: E402

N_CORES = 8
H, W = 256, 384
N_PIX = H * W                      # 98304
PIX_PER_CORE = N_PIX // N_CORES    # 12288
P = 128                            # partitions
F = PIX_PER_CORE // P              # 96 free-dim pixels per partition
C = 5                              # classes
EPS = 1e-8

F32 = mybir.dt.float32
BF16 = mybir.dt.bfloat16
Alu = mybir.AluOpType
ActFn = mybir.ActivationFunctionType
AxX = mybir.AxisListType.X

BF = ml_dtypes.bfloat16

# Set by callers that want a profile; results stashed in LAST_RESULTS.
TRACE = False
LAST_RESULTS = None

_PROGRAM_CACHE = {}


def _build_program(cw_adj):
    """Build + compile the per-core Bass program. The 5 (zero-replaced)
    class weights are baked in as immediates."""
    cw0 = float(cw_adj[0])
    nc = bacc.Bacc(
        "TRN2",
        target_bir_lowering=False,
        debug=False,
        enable_asserts=False,
        num_devices=N_CORES,
    )

    # pred: [128, 480] bf16 class-blocked (col c*96 + j); goldw: gold
    # class-blocked 480 cols then weight 96 cols, bf16.
    pred_d = nc.dram_tensor("pred", [P, C * F], BF16, kind="ExternalInput").ap()
    goldw_d = nc.dram_tensor(
        "goldw", [P, C * F + F], BF16, kind="ExternalInput"
    ).ap()
    acc_d = nc.dram_tensor("acc", [P, 1], F32, kind="ExternalOutput").ap()

    with tile.TileContext(nc) as tc:
        with tc.tile_pool(name="main", bufs=1) as pool:
            # --- input DMAs first so desc-gen leads each queue's stream
            p_t = pool.tile([P, C * F], BF16)
            nc.sync.dma_start(out=p_t[:], in_=pred_d)
            gw_t = pool.tile([P, C * F + F], BF16)
            nc.scalar.dma_start(out=gw_t[:], in_=goldw_d)

            # --- constants, built while the DMAs are in flight
            eps_t = pool.tile([P, 1], F32)
            nc.vector.memset(eps_t[:], EPS)

            # class-blocked weight pattern (block c -> cw[c])
            cwpat_t = pool.tile([P, C * F], BF16)
            for c in range(C):
                nc.vector.memset(
                    cwpat_t[:, c * F : (c + 1) * F], float(cw_adj[c])
                )

            # Warm up the ACT ln table before the input DMAs land.
            warm = pool.tile([P, 1], F32)
            nc.vector.memset(warm[:], 1.0)
            nc.scalar.activation(warm[:], warm[:], ActFn.Ln, bias=eps_t[:])

            # class-block views
            def blk(t, c):
                return t[:, c * F : (c + 1) * F]

            # --- pred chain: m = max_c pred via contiguous TT tree (exact)
            t1_t = pool.tile([P, F], BF16)
            nc.vector.tensor_tensor(t1_t[:], blk(p_t[:], 0), blk(p_t[:], 1),
                                    op=Alu.max)
            t2_t = pool.tile([P, F], BF16)
            nc.vector.tensor_tensor(t2_t[:], blk(p_t[:], 2), blk(p_t[:], 3),
                                    op=Alu.max)
            t3_t = pool.tile([P, F], BF16)
            nc.vector.tensor_tensor(t3_t[:], t1_t[:], t2_t[:], op=Alu.max)
            m_t = pool.tile([P, F], BF16)
            nc.vector.tensor_tensor(m_t[:], t3_t[:], blk(p_t[:], 4), op=Alu.max)

            # eq = (pred == m) -> bf16, broadcast m over the 5 blocks
            eq_t = pool.tile([P, C * F], BF16)
            eq_b = eq_t[:].rearrange("p (c j) -> p c j", c=C)
            m_b = m_t[:].unsqueeze(1).broadcast_to([P, C, F])
            p_b = p_t[:].rearrange("p (c j) -> p c j", c=C)
            nc.vector.tensor_tensor(eq_b, p_b, m_b, op=Alu.is_equal)

            # cwe = eq * cwpat (bf16 2x, contiguous)
            cwe_t = pool.tile([P, C * F], BF16)
            nc.vector.tensor_tensor(cwe_t[:], eq_t[:], cwpat_t[:], op=Alu.mult)

            # cwsel = sum_c cwe via TT tree (bf16)
            s1_t = pool.tile([P, F], BF16)
            nc.vector.tensor_tensor(s1_t[:], blk(cwe_t[:], 0), blk(cwe_t[:], 1),
                                    op=Alu.add)
            s2_t = pool.tile([P, F], BF16)
            nc.vector.tensor_tensor(s2_t[:], blk(cwe_t[:], 2), blk(cwe_t[:], 3),
                                    op=Alu.add)
            s3_t = pool.tile([P, F], BF16)
            nc.vector.tensor_tensor(s3_t[:], s1_t[:], s2_t[:], op=Alu.add)
            cwsel_t = pool.tile([P, F], BF16)
            nc.vector.tensor_tensor(cwsel_t[:], s3_t[:], blk(cwe_t[:], 4),
                                    op=Alu.add)

            # --- gold chain
            g_t = gw_t[:, 0 : C * F]
            w_v = gw_t[:, C * F : C * F + F]

            # gr = max(g1..g4) via TT tree
            r1_t = pool.tile([P, F], BF16)
            nc.vector.tensor_tensor(r1_t[:], blk(g_t, 1), blk(g_t, 2),
                                    op=Alu.max)
            r2_t = pool.tile([P, F], BF16)
            nc.vector.tensor_tensor(r2_t[:], blk(g_t, 3), blk(g_t, 4),
                                    op=Alu.max)
            gr_t = pool.tile([P, F], BF16)
            nc.vector.tensor_tensor(gr_t[:], r1_t[:], r2_t[:], op=Alu.max)

            # gmask = gr > g0 (strict gt == first-max-tie semantics), bf16
            gmask_t = pool.tile([P, F], BF16)
            nc.vector.tensor_tensor(gmask_t[:], gr_t[:], blk(g_t, 0),
                                    op=Alu.is_gt)

            # --- log-sum chain
            # L = ln(pred + eps) -> bf16 on ACT
            L_t = pool.tile([P, C * F], BF16)
            nc.scalar.activation(L_t[:], p_t[:], ActFn.Ln, bias=eps_t[:])

            # prod = gold * L (bf16 2x)
            prod_t = pool.tile([P, C * F], BF16)
            nc.vector.tensor_tensor(prod_t[:], g_t, L_t[:], op=Alu.mult)

            # u = sum_c prod via TT tree (bf16)
            v1_t = pool.tile([P, F], BF16)
            nc.vector.tensor_tensor(v1_t[:], blk(prod_t[:], 0),
                                    blk(prod_t[:], 1), op=Alu.add)
            v2_t = pool.tile([P, F], BF16)
            nc.vector.tensor_tensor(v2_t[:], blk(prod_t[:], 2),
                                    blk(prod_t[:], 3), op=Alu.add)
            v3_t = pool.tile([P, F], BF16)
            nc.vector.tensor_tensor(v3_t[:], v1_t[:], v2_t[:], op=Alu.add)
            u_t = pool.tile([P, F], BF16)
            nc.vector.tensor_tensor(u_t[:], v3_t[:], blk(prod_t[:], 4),
                                    op=Alu.add)

            # --- combine: loss_pix = u * (w + cw0*gmask + (1-gmask)*cwsel)
            base_t = pool.tile([P, F], BF16)
            nc.vector.scalar_tensor_tensor(
                base_t[:], gmask_t[:], cw0, w_v,
                op0=Alu.mult, op1=Alu.add,
            )
            tm_t = pool.tile([P, F], BF16)
            nc.vector.scalar_tensor_tensor(
                tm_t[:], gmask_t[:], 1.0, cwsel_t[:],
                op0=Alu.subtract, op1=Alu.mult,
            )
            tot_t = pool.tile([P, F], BF16)
            nc.vector.tensor_tensor(tot_t[:], base_t[:], tm_t[:],
                                    op=Alu.subtract)
            lp_t = pool.tile([P, F], BF16)
            nc.vector.tensor_tensor(lp_t[:], u_t[:], tot_t[:], op=Alu.mult)

            # acc = sum_j loss_pix  [128,1] f32
            acc_t = pool.tile([P, 1], F32)
            nc.vector.tensor_reduce(acc_t[:], lp_t[:], axis=AxX, op=Alu.add)

            nc.sync.dma_start(out=acc_d, in_=acc_t[:])

    nc.compile()
    return nc


def _cblock(arr5: np.ndarray, core: int) -> np.ndarray:
    """arr5: [5, 98304] -> per-core [128, 480] class-blocked
    (col c*96 + j)."""
    chunk = arr5[:, core * PIX_PER_CORE : (core + 1) * PIX_PER_CORE]
    # [5, 128, 96] -> [128, 5, 96] -> [128, 480]
    return chunk.reshape(C, P, F).transpose(1, 0, 2).reshape(P, C * F)


def kernel(pred, gold, weight, clss_weight_list):
    global LAST_RESULTS

    pred = np.asarray(pred, dtype=np.float32)
    gold = np.asarray(gold, dtype=np.float32)
    weight = np.asarray(weight, dtype=np.float32)
    cw = np.asarray(clss_weight_list, dtype=np.float32)[0]  # [5]
    cw_adj = np.where(cw == 0, cw[0], cw).astype(np.float32)

    key = cw_adj.tobytes()
    nc = _PROGRAM_CACHE.get(key)
    if nc is None:
        nc = _build_program([float(x) for x in cw_adj])
        _PROGRAM_CACHE[key] = nc

    p5 = pred[0].reshape(C, N_PIX).astype(BF)
    g5 = gold[0].reshape(C, N_PIX).astype(BF)
    w1 = weight[0].reshape(N_PIX).astype(BF)

    in_maps = []
    for k in range(N_CORES):
        gw = np.empty((P, C * F + F), dtype=BF)
        gw[:, 0 : C * F] = _cblock(g5, k)
        gw[:, C * F :] = w1[k * PIX_PER_CORE : (k + 1) * PIX_PER_CORE].reshape(
            P, F
        )
        in_maps.append(
            {
                "pred": np.ascontiguousarray(_cblock(p5, k)),
                "goldw": gw,
            }
        )

    res = run_bass_kernel_spmd(
        nc, in_maps, list(range(N_CORES)), trace=TRACE
    )
    LAST_RESULTS = res

    total = 0.0
    for k in range(N_CORES):
        total += float(np.asarray(res.results[k]["acc"], dtype=np.float64).sum())

    loss = -total / N_PIX
    return np.float32(loss)


# revision 8
# speedup vs baseline: 1.3281x; 1.3281x over previous
"""Trainium2 Bass kernel for nn_CrossEntropyLoss_2585570312585.

Reference computation (jax):
    cw = where(cw == 0, cw[0], cw)                      # [5]
    gold2dim   = argmax(gold, axis=class)               # [256,384]
    prediction = argmax(pred, axis=class)
    pred_fp    = where(gold2dim > 0, 0,
                       where(prediction == gold2dim, 0, prediction))
    weight_fp  = cw[pred_fp]
    loss = -(weight + weight_fp) * sum_c(gold * log(pred + 1e-8))
    out  = mean(loss)                                   # scalar

Restructuring (identical up to fp rounding / measure-zero ties):
    u      = sum_c gold_c * ln(pred_c + eps)
    gmask  = max(g1..g4) > g0            (== gold2dim > 0, first-max ties ok)
    cwsel  = sum_c cw_c * (p_c == max_c p_c)   (== cw[argmax p] up to ties)
    loss_pix = u * (w + cw0*gmask + (1-gmask)*cwsel)
    out = -mean(loss_pix)

Perf design (v3), from HW microbenchmarks of this toolchain:
  * ~16.7 us of the 19.3 us v1 baseline is fixed NEFF overhead. The
    controllable part: input-DMA latency, DVE compute, output-DMA tail.
  * All inputs bf16 (host converts; free for HW time). Measured end-to-end
    rel err of the full bf16 scheme: 2.0e-3 (gate 2e-2).
  * Inputs CLASS-BLOCKED per partition: [128, 5 blocks x 96]. Measured DVE
    costs: tensor_reduce 480->96 = 561 ns regardless of dtype (input-bound
    ~110 elem/ns), but plain bf16 tensor_tensor [128,96] = 116 ns. So every
    class reduction becomes a tree of contiguous TT ops (464 ns vs 561,
    and 348 vs 458+ for the 4-class max), and is_equal runs without the
    broadcast penalty path where possible.
  * tensor_tensor_reduce is broken on this toolchain (compiles, wedges the
    device: NRT_EXEC_UNIT_UNRECOVERABLE). Do not use.
  * Two parallel HWDGE queues: pred on qSP (Sync), gold+weight on
    qActivation (Scalar). No SWDGE (gpsimd) DMA.
  * cw baked as immediates into a class-blocked [128,480] bf16 pattern
    (5 contiguous memsets, issued pre-data -> free).
  * Output MUST be a single descriptor: a [128,1] output (128 tiny
    descriptors over 16 DMA rings) hits a pathological completion path --
    the 16 per-ring completion semaphores crawl in over ~7 us (measured;
    same effect makes an empty kernel cost 16.7 us). So the final
    partition sum runs on the idle TensorE (ones^T @ lp -> PSUM [1,96]),
    a 1-partition reduce makes [1,1], and the output DMA is 1 descriptor.

Sharding: the 256x384 = 98304-pixel plane splits into 8 contiguous
chunks of 12288 pixels (one per NeuronCore), laid out [128 x 96] per
class block.
"""

import os
import sys

import numpy as np


def _ensure_concourse():
    try:
        import concourse  # noqa: F401
        return
    except ImportError:
        pass
    for p in ("/opt/trn_rl_repo", "/root/.axon_site/_ro/trn_rl_repo"):
        if os.path.isdir(p) and p not in sys.path:
            sys.path.insert(0, p)
    import concourse  # noqa: F401


_ensure_concourse()

import ml_dtypes  # noqa: E402

import concourse.bass as bass  # noqa: E402
import concourse.tile as tile  # noqa: E402
from concourse import bacc, mybir  # noqa: E402
from concourse.bass_utils import run_bass_kernel_spmd  # noqa: E402

N_CORES = 8
H, W = 256, 384
N_PIX = H * W                      # 98304
PIX_PER_CORE = N_PIX // N_CORES    # 12288
P = 128                            # partitions
F = PIX_PER_CORE // P              # 96 free-dim pixels per partition
C = 5                              # classes
EPS = 1e-8

F32 = mybir.dt.float32
BF16 = mybir.dt.bfloat16
Alu = mybir.AluOpType
ActFn = mybir.ActivationFunctionType
AxX = mybir.AxisListType.X

BF = ml_dtypes.bfloat16

# Set by callers that want a profile; results stashed in LAST_RESULTS.
TRACE = False
LAST_RESULTS = None

_PROGRAM_CACHE = {}


def _build_program(cw_adj):
    """Build + compile the per-core Bass program. The 5 (zero-replaced)
    class weights are baked in as immediates."""
    cw0 = float(cw_adj[0])
    nc = bacc.Bacc(
        "TRN2",
        target_bir_lowering=False,
        debug=False,
        enable_asserts=False,
        num_devices=N_CORES,
    )

    # pred: [128, 480] bf16 class-blocked (col c*96 + j); goldw: gold
    # class-blocked 480 cols then weight 96 cols, bf16.
    pred_d = nc.dram_tensor("pred", [P, C * F], BF16, kind="ExternalInput").ap()
    goldw_d = nc.dram_tensor(
        "goldw", [P, C * F + F], BF16, kind="ExternalInput"
    ).ap()
    out_d = nc.dram_tensor("out", [1, 1], F32, kind="ExternalOutput").ap()

    with tile.TileContext(nc) as tc:
        with (
            tc.tile_pool(name="main", bufs=1) as pool,
            tc.tile_pool(name="psum", bufs=1, space="PSUM") as psum_pool,
        ):
            # --- input DMAs first so desc-gen leads each queue's stream
            p_t = pool.tile([P, C * F], BF16)
            nc.sync.dma_start(out=p_t[:], in_=pred_d)
            gw_t = pool.tile([P, C * F + F], BF16)
            nc.scalar.dma_start(out=gw_t[:], in_=goldw_d)

            # --- constants, built while the DMAs are in flight
            eps_t = pool.tile([P, 1], F32)
            nc.vector.memset(eps_t[:], EPS)

            # class-blocked weight pattern (block c -> cw[c])
            cwpat_t = pool.tile([P, C * F], BF16)
            for c in range(C):
                nc.vector.memset(
                    cwpat_t[:, c * F : (c + 1) * F], float(cw_adj[c])
                )

            # ones column for the TensorE partition sum
            ones_t = pool.tile([P, 1], BF16)
            nc.vector.memset(ones_t[:], 1.0)

            # class-block views
            def blk(t, c):
                return t[:, c * F : (c + 1) * F]

            # --- pred chain: m = max_c pred via contiguous TT tree (exact)
            t1_t = pool.tile([P, F], BF16)
            nc.vector.tensor_tensor(t1_t[:], blk(p_t[:], 0), blk(p_t[:], 1),
                                    op=Alu.max)
            t2_t = pool.tile([P, F], BF16)
            nc.vector.tensor_tensor(t2_t[:], blk(p_t[:], 2), blk(p_t[:], 3),
                                    op=Alu.max)
            t3_t = pool.tile([P, F], BF16)
            nc.vector.tensor_tensor(t3_t[:], t1_t[:], t2_t[:], op=Alu.max)
            m_t = pool.tile([P, F], BF16)
            nc.vector.tensor_tensor(m_t[:], t3_t[:], blk(p_t[:], 4), op=Alu.max)

            # eq = (pred == m) -> bf16, broadcast m over the 5 blocks
            eq_t = pool.tile([P, C * F], BF16)
            eq_b = eq_t[:].rearrange("p (c j) -> p c j", c=C)
            m_b = m_t[:].unsqueeze(1).broadcast_to([P, C, F])
            p_b = p_t[:].rearrange("p (c j) -> p c j", c=C)
            nc.vector.tensor_tensor(eq_b, p_b, m_b, op=Alu.is_equal)

            # cwe = eq * cwpat (bf16 2x, contiguous)
            cwe_t = pool.tile([P, C * F], BF16)
            nc.vector.tensor_tensor(cwe_t[:], eq_t[:], cwpat_t[:], op=Alu.mult)

            # cwsel = sum_c cwe via TT tree (bf16)
            s1_t = pool.tile([P, F], BF16)
            nc.vector.tensor_tensor(s1_t[:], blk(cwe_t[:], 0), blk(cwe_t[:], 1),
                                    op=Alu.add)
            s2_t = pool.tile([P, F], BF16)
            nc.vector.tensor_tensor(s2_t[:], blk(cwe_t[:], 2), blk(cwe_t[:], 3),
                                    op=Alu.add)
            s3_t = pool.tile([P, F], BF16)
            nc.vector.tensor_tensor(s3_t[:], s1_t[:], s2_t[:], op=Alu.add)
            cwsel_t = pool.tile([P, F], BF16)
            nc.vector.tensor_tensor(cwsel_t[:], s3_t[:], blk(cwe_t[:], 4),
                                    op=Alu.add)

            # --- gold chain
            g_t = gw_t[:, 0 : C * F]
            w_v = gw_t[:, C * F : C * F + F]

            # gr = max(g1..g4) via TT tree
            r1_t = pool.tile([P, F], BF16)
            nc.vector.tensor_tensor(r1_t[:], blk(g_t, 1), blk(g_t, 2),
                                    op=Alu.max)
            r2_t = pool.tile([P, F], BF16)
            nc.vector.tensor_tensor(r2_t[:], blk(g_t, 3), blk(g_t, 4),
                                    op=Alu.max)
            gr_t = pool.tile([P, F], BF16)
            nc.vector.tensor_tensor(gr_t[:], r1_t[:], r2_t[:], op=Alu.max)

            # gmask = gr > g0 (strict gt == first-max-tie semantics), bf16
            gmask_t = pool.tile([P, F], BF16)
            nc.vector.tensor_tensor(gmask_t[:], gr_t[:], blk(g_t, 0),
                                    op=Alu.is_gt)

            # --- log-sum chain
            # L = ln(pred + eps) -> bf16 on ACT
            L_t = pool.tile([P, C * F], BF16)
            nc.scalar.activation(L_t[:], p_t[:], ActFn.Ln, bias=eps_t[:])

            # prod = gold * L (bf16 2x)
            prod_t = pool.tile([P, C * F], BF16)
            nc.vector.tensor_tensor(prod_t[:], g_t, L_t[:], op=Alu.mult)

            # u = sum_c prod via TT tree (bf16)
            v1_t = pool.tile([P, F], BF16)
            nc.vector.tensor_tensor(v1_t[:], blk(prod_t[:], 0),
                                    blk(prod_t[:], 1), op=Alu.add)
            v2_t = pool.tile([P, F], BF16)
            nc.vector.tensor_tensor(v2_t[:], blk(prod_t[:], 2),
                                    blk(prod_t[:], 3), op=Alu.add)
            v3_t = pool.tile([P, F], BF16)
            nc.vector.tensor_tensor(v3_t[:], v1_t[:], v2_t[:], op=Alu.add)
            u_t = pool.tile([P, F], BF16)
            nc.vector.tensor_tensor(u_t[:], v3_t[:], blk(prod_t[:], 4),
                                    op=Alu.add)

            # --- combine: loss_pix = u * (w + cw0*gmask + (1-gmask)*cwsel)
            base_t = pool.tile([P, F], BF16)
            nc.vector.scalar_tensor_tensor(
                base_t[:], gmask_t[:], cw0, w_v,
                op0=Alu.mult, op1=Alu.add,
            )
            tm_t = pool.tile([P, F], BF16)
            nc.vector.scalar_tensor_tensor(
                tm_t[:], gmask_t[:], 1.0, cwsel_t[:],
                op0=Alu.subtract, op1=Alu.mult,
            )
            tot_t = pool.tile([P, F], BF16)
            nc.vector.tensor_tensor(tot_t[:], base_t[:], tm_t[:],
                                    op=Alu.subtract)
            lp_t = pool.tile([P, F], BF16)
            nc.vector.tensor_tensor(lp_t[:], u_t[:], tot_t[:], op=Alu.mult)

            # cross-partition sum on TensorE: psum[1,96] = ones^T @ lp
            ps_t = psum_pool.tile([1, F], F32)
            nc.tensor.matmul(ps_t[:], ones_t[:], lp_t[:])

            # final 96-col reduce on one partition -> [1,1] sbuf
            out_t = pool.tile([1, 1], F32)
            nc.vector.tensor_reduce(out_t[:], ps_t[:], axis=AxX, op=Alu.add)

            nc.sync.dma_start(out=out_d, in_=out_t[:])

    nc.compile()
    return nc


def _cblock(arr5: np.ndarray, core: int) -> np.ndarray:
    """arr5: [5, 98304] -> per-core [128, 480] class-blocked
    (col c*96 + j)."""
    chunk = arr5[:, core * PIX_PER_CORE : (core + 1) * PIX_PER_CORE]
    # [5, 128, 96] -> [128, 5, 96] -> [128, 480]
    return chunk.reshape(C, P, F).transpose(1, 0, 2).reshape(P, C * F)


def kernel(pred, gold, weight, clss_weight_list):
    global LAST_RESULTS

    pred = np.asarray(pred, dtype=np.float32)
    gold = np.asarray(gold, dtype=np.float32)
    weight = np.asarray(weight, dtype=np.float32)
    cw = np.asarray(clss_weight_list, dtype=np.float32)[0]  # [5]
    cw_adj = np.where(cw == 0, cw[0], cw).astype(np.float32)

    key = cw_adj.tobytes()
    nc = _PROGRAM_CACHE.get(key)
    if nc is None:
        nc = _build_program([float(x) for x in cw_adj])
        _PROGRAM_CACHE[key] = nc

    p5 = pred[0].reshape(C, N_PIX).astype(BF)
    g5 = gold[0].reshape(C, N_PIX).astype(BF)
    w1 = weight[0].reshape(N_PIX).astype(BF)

    in_maps = []
    for k in range(N_CORES):
        gw = np.empty((P, C * F + F), dtype=BF)
        gw[:, 0 : C * F] = _cblock(g5, k)
        gw[:, C * F :] = w1[k * PIX_PER_CORE : (k + 1) * PIX_PER_CORE].reshape(
            P, F
        )
        in_maps.append(
            {
                "pred": np.ascontiguousarray(_cblock(p5, k)),
                "goldw": gw,
            }
        )

    res = run_bass_kernel_spmd(
        nc, in_maps, list(range(N_CORES)), trace=TRACE
    )
    LAST_RESULTS = res

    total = 0.0
    for k in range(N_CORES):
        total += float(np.asarray(res.results[k]["out"])[0, 0])

    loss = -total / N_PIX
    return np.float32(loss)
